# revision 4
# baseline (speedup 1.0000x reference)
"""NT-Xent (SimCLR contrastive) loss on Trainium2, sharded across 8 NeuronCores.

Each core computes a [512, 4096] row-slice of the similarity matrix
sim = zn_own^T . zn_all (fp8 DoubleRow matmuls, x16 fp8 scaling), with the
exp row-sums fused into ScalarE's activation accumulator, an exact fp8-level
diagonal recompute, and positives from a host-shipped partner slice. Host
sums the 8 scalar partials (the unshard step). No host arithmetic beyond
sharding/layout/dtype-cast of inputs and summing the per-core partials.

v2 schedule (vs the 62.7us baseline):
  - DMA order zok, zb0h0, zb0h1, zb1h0, zb1h1, zb2, zb3, zpk: the own-row
    chain (which gates the Gram lhs) starts first; the partner slice, only
    needed for the finale, lands last.
  - blocks 0/1 flow through the normalize conveyor in 512-column halves so
    the first Gram PSUM tile closes ~10us earlier than whole-block flow.
  - engine rebalance: squares for blocks 2/3 and the pos/diag products run
    on GpSimd (idle in the baseline); DVE keeps the 2x-mode bf16 multiplies;
    per-engine emission order is chosen so no conveyor stage queues behind
    a later-dependency instruction (the baseline's DVE queue serialized the
    first fp8 block to ~28us).
  - one activation-table load: Ln/Exp pinned via the bacc table-map patch.
  - PE warmers bridge the input-DMA head so the clock gate is at full rate
    when the ssq/Gram matmuls arrive.
"""

import numpy as np

B = 2048
D = 512
N2 = 2 * B              # 4096 total rows
NCORES = 8
RPC = N2 // NCORES      # 512 rows per core
KT = D // 128           # 4 contraction tiles
BLK = 1024              # column-block size
NBLK = N2 // BLK        # 4 blocks
TEMP = 0.1
SCALE = 1.0 / TEMP      # 10.0
FP8_SCALE = 16.0        # zn is stored as fp8(zn*16); sim256 = 256*sim
LN_FP8 = float(np.log(FP8_SCALE))
NWARM_A = 24            # PE warmers during the first DMA wait
NWARM_B = 10            # second batch after the own-slice ssq

_CACHE = {}


def _build():
    from concourse import bass, bacc, tile, mybir

    nc = bacc.Bacc("TRN2", target_bir_lowering=False, debug=False,
                   num_devices=NCORES)
    bf16 = mybir.dt.bfloat16
    f32 = mybir.dt.float32
    f8 = mybir.dt.float8e4
    F = mybir.ActivationFunctionType
    A = mybir.AluOpType
    AX = mybir.AxisListType
    DR = mybir.MatmulPerfMode.DoubleRow
    PSUM = bass.MemorySpace.PSUM

    # host-pre-permuted: per-partition-contiguous layouts for fast DMA
    zt = nc.dram_tensor("zt", [128, NBLK, KT, BLK], bf16,
                        kind="ExternalInput").ap()
    zown = nc.dram_tensor("zown", [128, KT, RPC], bf16,
                          kind="ExternalInput").ap()
    zpr = nc.dram_tensor("zpr", [128, KT, RPC], bf16,
                         kind="ExternalInput").ap()
    out = nc.dram_tensor("out", [1, 1], f32, kind="ExternalOutput").ap()

    with tile.TileContext(nc) as tc:
        with (
            tc.tile_pool(name="sb", bufs=1) as sb,
            tc.tile_pool(name="wrk", bufs=2) as wrk,
            tc.tile_pool(name="wrk1", bufs=1) as wrk1,
            tc.tile_pool(name="psN", bufs=1, space=PSUM) as psN,
            tc.tile_pool(name="psO", bufs=2, space=PSUM) as psO,
            tc.tile_pool(name="psG", bufs=2, space=PSUM) as psG,
        ):
            ones = sb.tile([128, 128], bf16, tag="ones")
            nc.vector.memset(ones[:], 1.0)
            bias_ln16 = sb.tile([128, 1], f32, tag="b16")
            nc.vector.memset(bias_ln16[:], LN_FP8)
            bias_10 = sb.tile([128, 1], f32, tag="b10")
            nc.vector.memset(bias_10[:], SCALE)

            # ---- input DMAs, all on the sync HWDGE queue.  Own slice
            # first (gates the Gram lhs), partner slice last (finale only).
            zok = sb.tile([128, KT, RPC], bf16, tag="zok")
            zpk = sb.tile([128, KT, RPC], bf16, tag="zpk")
            zb = [sb.tile([128, KT, BLK], bf16, tag=f"zt{b}", name=f"zb{b}")
                  for b in range(NBLK)]
            nc.sync.dma_start(out=zok[:], in_=zown)
            nc.sync.dma_start(out=zb[0][:, :, 0:512], in_=zt[:, 0, :, 0:512])
            nc.sync.dma_start(out=zb[0][:, :, 512:1024], in_=zt[:, 0, :, 512:1024])
            nc.sync.dma_start(out=zb[1][:, :, 0:512], in_=zt[:, 1, :, 0:512])
            nc.sync.dma_start(out=zb[1][:, :, 512:1024], in_=zt[:, 1, :, 512:1024])
            nc.sync.dma_start(out=zb[2][:], in_=zt[:, 2])
            nc.sync.dma_start(out=zb[3][:], in_=zt[:, 3])
            nc.sync.dma_start(out=zpk[:], in_=zpr)

            # ---- PE warmers: keep the clock gate up through the head
            warm = psO.tile([128, 512], f32, tag="pd", name="warmA")
            for _ in range(NWARM_A):
                nc.tensor.matmul(warm[:, 0:128], ones[:], ones[:],
                                 start=True, stop=True)

            # ================= own-slice chain (Gram lhs) =================
            sqo = wrk.tile([128, KT, RPC], bf16, tag="sq_s", name="sqo")
            nc.vector.tensor_tensor(sqo[:], zok[:], zok[:], A.mult)
            pso = psO.tile([128, 512], f32, tag="pd", name="pso")
            for k in range(KT):
                nc.tensor.matmul(pso[:], ones[:], sqo[:, k, :],
                                 start=(k == 0), stop=(k == KT - 1))
            warm2 = psO.tile([128, 512], f32, tag="pd", name="warmB")
            for _ in range(NWARM_B):
                nc.tensor.matmul(warm2[:, 0:128], ones[:], ones[:],
                                 start=True, stop=True)
            lno = wrk.tile([128, RPC], f32, tag="lns_s", name="lno")
            nc.scalar.activation(lno[:], pso[:], F.Ln)
            rino = wrk.tile([128, RPC], bf16, tag="rin_s", name="rino")
            nc.scalar.activation(rino[:], lno[:], F.Exp, scale=-0.5,
                                 bias=bias_ln16[:])
            zno = sb.tile([128, KT, RPC], f8, tag="zno")
            nc.vector.tensor_tensor(
                zno[:], zok[:],
                rino[:].unsqueeze(1).broadcast_to([128, KT, RPC]), A.mult)

            # ================= block normalize conveyor ===================
            zn16 = [sb.tile([128, KT, BLK], bf16, tag=f"zn16_{b}",
                            name=f"zn16_{b}") for b in range(NBLK)]
            zn8 = [sb.tile([128, KT, BLK], f8, tag=f"zn8_{b}",
                           name=f"zn8_{b}") for b in range(NBLK)]
            rin = [None] * NBLK
            psS = [None] * NBLK

            def ssq_half(b, h):
                """ones-matmul partial ssq of columns [512h, 512h+512)."""
                for k in range(KT):
                    nc.tensor.matmul(psS[b][:, h * 512:(h + 1) * 512],
                                     ones[:], sq_t[b][:, k, h * 512:(h + 1) * 512],
                                     start=(k == 0), stop=(k == KT - 1))

            def rsqrt_block(b):
                lns = wrk.tile([128, BLK], f32, tag="lns", name=f"lns{b}")
                nc.scalar.activation(lns[:], psS[b][:], F.Ln)
                rin[b] = wrk1.tile([128, BLK], bf16, tag=f"rin{b}",
                                   name=f"rin{b}")
                nc.scalar.activation(rin[b][:], lns[:], F.Exp, scale=-0.5,
                                     bias=bias_ln16[:])

            def mult_half(b, h):
                s = slice(h * 512, (h + 1) * 512)
                nc.vector.tensor_tensor(
                    zn16[b][:, :, s], zb[b][:, :, s],
                    rin[b][:, s].unsqueeze(1).broadcast_to([128, KT, 512]),
                    A.mult)
                nc.gpsimd.dma_start(out=zn8[b][:, :, s], in_=zn16[b][:, :, s])

            def mult_full(b):
                nc.vector.tensor_tensor(
                    zn16[b][:], zb[b][:],
                    rin[b][:].unsqueeze(1).broadcast_to([128, KT, BLK]),
                    A.mult)
                nc.gpsimd.dma_start(out=zn8[b][:], in_=zn16[b][:])

            # squares: blocks 0/1 on DVE (2x mode, halves), 2/3 on GpSimd
            sq_t = [None] * NBLK
            sq_t[0] = wrk.tile([128, KT, BLK], bf16, tag="sq01", name="sq0")
            sq_t[1] = wrk.tile([128, KT, BLK], bf16, tag="sq01", name="sq1")
            sq_t[2] = wrk.tile([128, KT, BLK], bf16, tag="sq23", name="sq2")
            sq_t[3] = wrk.tile([128, KT, BLK], bf16, tag="sq23", name="sq3")

            psS[0] = psN.tile([128, BLK], f32, tag="ssq", name="psS0")
            psS[1] = psN.tile([128, BLK], f32, tag="ssq", name="psS1")
            psS[2] = psN.tile([128, BLK], f32, tag="ssq", name="psS2")
            psS[3] = psN.tile([128, BLK], f32, tag="ssq", name="psS3")

            # DVE: squares of block 0 halves, then block 1 halves
            for b in (0, 1):
                for h in (0, 1):
                    s = slice(h * 512, (h + 1) * 512)
                    nc.vector.tensor_tensor(sq_t[b][:, :, s],
                                            zb[b][:, :, s], zb[b][:, :, s],
                                            A.mult)
            # GpSimd: squares of blocks 2/3 (whole)
            nc.gpsimd.tensor_tensor(sq_t[2][:], zb[2][:], zb[2][:], A.mult)
            nc.gpsimd.tensor_tensor(sq_t[3][:], zb[3][:], zb[3][:], A.mult)

            # PE: ssq accumulations (after own-ssq + warmers, before Gram)
            ssq_half(0, 0)
            ssq_half(0, 1)
            ssq_half(1, 0)
            ssq_half(1, 1)

            # ScalarE: rsqrt factors for blocks 0/1 (queued after own Ln/Exp)
            rsqrt_block(0)
            rsqrt_block(1)

            # DVE: the normalize multiplies + SWDGE fp8 casts
            mult_half(0, 0)
            mult_half(0, 1)
            mult_half(1, 0)
            mult_half(1, 1)

            # ---------- Gram + fused exp row-sums ----------
            rowp = sb.tile([128, 4, NBLK], f32, tag="rowp")
            scr_n = [0]

            def gram_group(b, m):
                pm = psG.tile([128, BLK], f32, tag="mm", name=f"pm{b}_{m}")
                lhsT0 = zno[:, 0:2, m * 128:(m + 1) * 128]
                lhsT1 = zno[:, 2:4, m * 128:(m + 1) * 128]
                for j in range(2):
                    js = slice(j * 512, (j + 1) * 512)
                    nc.tensor.matmul(pm[:, js], lhsT0, zn8[b][:, 0:2, js],
                                     start=True, stop=False, perf_mode=DR)
                    nc.tensor.matmul(pm[:, js], lhsT1, zn8[b][:, 2:4, js],
                                     start=False, stop=True, perf_mode=DR)
                scr = wrk.tile([128, BLK], bf16, tag="scr",
                               name=f"scr{scr_n[0]}")
                scr_n[0] += 1
                nc.scalar.activation(
                    scr[:], pm[:], F.Exp,
                    scale=SCALE / (FP8_SCALE ** 2),
                    accum_out=rowp[:, m, b:b + 1])

            # block 0 Gram (PE) + exps (ScalarE)
            for m in range(4):
                gram_group(0, m)

            # block 2 conveyor pieces (emitted so they slot between groups)
            ssq_half(2, 0)
            ssq_half(2, 1)
            rsqrt_block(2)
            mult_full(2)

            for m in range(4):
                gram_group(1, m)

            ssq_half(3, 0)
            ssq_half(3, 1)
            rsqrt_block(3)
            mult_full(3)

            # ---- partner norm (bf16 x1) while the exp stream runs ----
            sqp = wrk.tile([128, KT, RPC], bf16, tag="sq_s", name="sqp")
            nc.vector.tensor_tensor(sqp[:], zpk[:], zpk[:], A.mult)
            psp = psO.tile([128, 512], f32, tag="pd", name="psp")
            for k in range(KT):
                nc.tensor.matmul(psp[:], ones[:], sqp[:, k, :],
                                 start=(k == 0), stop=(k == KT - 1))
            lnp = wrk.tile([128, RPC], f32, tag="lns_s", name="lnp")
            nc.scalar.activation(lnp[:], psp[:], F.Ln)
            rinp = wrk.tile([128, RPC], bf16, tag="rin_s", name="rinp")
            nc.scalar.activation(rinp[:], lnp[:], F.Exp, scale=-0.5)
            znp = sb.tile([128, KT, RPC], bf16, tag="znp")
            nc.vector.tensor_tensor(
                znp[:], zpk[:],
                rinp[:].unsqueeze(1).broadcast_to([128, KT, RPC]), A.mult)

            # ---- diagonal recompute (exact fp8-level) ----
            prd = wrk.tile([128, KT, RPC], bf16, tag="prod", name="prd")
            nc.gpsimd.tensor_tensor(prd[:], zno[:], zno[:], A.mult)
            dg = psO.tile([128, 512], f32, tag="pd", name="dg")
            for k in range(KT):
                nc.tensor.matmul(dg[0:1, :], ones[:, 0:1], prd[:, k, :],
                                 start=(k == 0), stop=(k == KT - 1))
            diag_row = sb.tile([1, RPC], bf16, tag="diagrow")
            nc.vector.tensor_scalar_add(diag_row[:], dg[0:1, :],
                                        -FP8_SCALE ** 2)
            dt = psO.tile([128, 512], f32, tag="pd", name="dt")
            for m in range(4):
                nc.tensor.matmul(dt[:, m * 128:(m + 1) * 128],
                                 diag_row[0:1, m * 128:(m + 1) * 128],
                                 ones[0:1, :], start=True, stop=True)
            diag_part = sb.tile([128, 4], f32, tag="diagp")
            for m in range(4):
                nc.vector.tensor_copy(diag_part[:, m:m + 1],
                                      dt[:, m * 128:m * 128 + 1])
            dexp = sb.tile([128, 4], f32, tag="dexp")
            nc.scalar.activation(dexp[:], diag_part[:], F.Exp,
                                 scale=SCALE / (FP8_SCALE ** 2),
                                 bias=bias_10[:])

            # late Gram blocks
            for m in range(4):
                gram_group(2, m)
            for m in range(4):
                gram_group(3, m)

            # ---- positives (fp8 own x bf16 partner) ----
            prp = wrk.tile([128, KT, RPC], bf16, tag="prod", name="prp")
            nc.gpsimd.tensor_tensor(prp[:], zno[:], znp[:], A.mult)
            pp = psO.tile([128, 512], f32, tag="pd", name="pp")
            for k in range(KT):
                nc.tensor.matmul(pp[:], ones[:], prp[:, k, :],
                                 start=(k == 0), stop=(k == KT - 1))
            pos_red = sb.tile([128, 1], f32, tag="posr")
            nc.vector.tensor_reduce(pos_red[:], pp[:], AX.X, A.add)

            # ---- finale: partial = sum_r ln(Z_r) - 10 * sum_r pos_r ----
            zs = sb.tile([128, 4], f32, tag="zs")
            nc.vector.tensor_reduce(zs[:], rowp[:], AX.X, A.add)
            zarg = sb.tile([128, 4], f32, tag="zarg")
            nc.vector.tensor_tensor(zarg[:], zs[:], dexp[:], A.subtract)
            logz = sb.tile([128, 5], f32, tag="logz")
            nc.scalar.activation(logz[:, 0:4], zarg[:], F.Ln)
            nc.vector.tensor_scalar_mul(
                logz[:, 4:5], pos_red[:], -SCALE / FP8_SCALE / 128.0)
            red1 = sb.tile([128, 1], f32, tag="red1")
            nc.vector.tensor_reduce(red1[:], logz[:], AX.X, A.add)
            fin = sb.tile([1, 1], f32, tag="fin")
            nc.gpsimd.tensor_reduce(fin[:], red1[:], AX.C, A.add)
            nc.sync.dma_start(out=out, in_=fin[:])

    from concourse import bacc as _bacc_mod

    orig_tables = _bacc_mod.get_activation_tables

    def _filtered(arch):
        tables = orig_tables(arch)
        keep = "natural_log_exp_and_others"
        F = mybir.ActivationFunctionType
        if (keep in tables and F.Exp in tables[keep]
                and F.Ln in tables[keep]):
            for name, fns in tables.items():
                if name != keep:
                    fns.discard(F.Exp)
                    fns.discard(F.Ln)
        return tables

    _bacc_mod.get_activation_tables = _filtered
    try:
        nc.compile()
    finally:
        _bacc_mod.get_activation_tables = orig_tables
    return nc


def _get_nc():
    if "nc" not in _CACHE:
        _CACHE["nc"] = _build()
    return _CACHE["nc"]


def _in_maps(z_i, z_j):
    import ml_dtypes

    z = np.concatenate(
        [np.asarray(z_i, np.float32), np.asarray(z_j, np.float32)], axis=0)
    zt = np.ascontiguousarray(z.T).astype(ml_dtypes.bfloat16)
    # [D, N2] -> [128(p), NBLK, KT, BLK]: per-partition contiguous
    ztH = np.ascontiguousarray(
        zt.reshape(KT, 128, NBLK, BLK).transpose(1, 2, 0, 3))

    def slc(off):
        s = zt[:, off:off + RPC]            # [D, RPC]
        return np.ascontiguousarray(s.reshape(KT, 128, RPC).transpose(1, 0, 2))

    maps = []
    for c in range(NCORES):
        o = c * RPC
        po = (o + B) % N2
        maps.append({
            "zt": ztH,
            "zown": slc(o),
            "zpr": slc(po),
        })
    return maps


def _run(z_i, z_j, trace=False):
    from concourse.bass_utils import run_bass_kernel_spmd

    nc = _get_nc()
    return run_bass_kernel_spmd(nc, _in_maps(z_i, z_j), list(range(NCORES)),
                                trace=trace)


def kernel(z_i, z_j):
    res = _run(z_i, z_j, trace=False)
    total = sum(float(r["out"][0, 0]) for r in res.results)
    return np.float32(total / N2)


# revision 5
# speedup vs baseline: 1.2179x; 1.2179x over previous
"""NT-Xent (SimCLR contrastive) loss on Trainium2, sharded across 8 NeuronCores.

Each core computes a [512, 4096] row-slice of the similarity matrix
sim = zn_own^T . zn_all (fp8 DoubleRow matmuls, x16 fp8 scaling), with the
exp row-sums fused into ScalarE's activation accumulator, an exact fp8-level
diagonal recompute, and positives from a host-shipped partner slice. Host
sums the 8 scalar partials (the unshard step). No host arithmetic beyond
sharding/layout/dtype-cast of inputs and summing the per-core partials.

v3 schedule (vs the 62.7us baseline):
  - DMA order zok, zb0h0, zb0h1, zb1h0, zb1h1, zb2, zpk, zb3: the own-row
    chain (which gates the Gram lhs) starts first; host layout is half-major
    [128, NBLK, 2, KT, 512] so every transfer is one 4-8KB descriptor per
    partition.
  - blocks 0/1 flow through the normalize conveyor in 512-column halves so
    the first Gram PSUM tile closes ~10us earlier than whole-block flow.
  - ALL bulk elementwise work stays on DVE (GpSimd tensor ops measure ~3.5x
    slower AND collapse concurrent DVE throughput ~6x via SBUF contention);
    GpSimd only triggers the SWDGE fp8 cast-DMAs and the final partition
    reduce.  DVE emission order is latency-sorted: each block's normalize
    multiply is queued before any later block's square.
  - one activation-table load: Ln/Exp pinned via the bacc table-map patch.
  - PE warmers bridge the input-DMA head so the clock gate is at full rate
    when the ssq/Gram matmuls arrive.
"""

import numpy as np

B = 2048
D = 512
N2 = 2 * B              # 4096 total rows
NCORES = 8
RPC = N2 // NCORES      # 512 rows per core
KT = D // 128            # 4 contraction tiles
BLK = 1024              # column-block size
NBLK = N2 // BLK        # 4 blocks
TEMP = 0.1
SCALE = 1.0 / TEMP      # 10.0
FP8_SCALE = 16.0        # zn is stored as fp8(zn*16); sim256 = 256*sim
LN_FP8 = float(np.log(FP8_SCALE))
NWARM_A = 24            # PE warmers during the first DMA wait
NWARM_B = 10            # second batch after the own-slice ssq

_CACHE = {}


def _build():
    from concourse import bass, bacc, tile, mybir

    nc = bacc.Bacc("TRN2", target_bir_lowering=False, debug=False,
                   num_devices=NCORES)
    bf16 = mybir.dt.bfloat16
    f32 = mybir.dt.float32
    f8 = mybir.dt.float8e4
    F = mybir.ActivationFunctionType
    A = mybir.AluOpType
    AX = mybir.AxisListType
    DR = mybir.MatmulPerfMode.DoubleRow
    PSUM = bass.MemorySpace.PSUM

    # host-pre-permuted, half-major: zt[p, b, h, k, j] = z^T's column
    # (b*1024 + h*512 + j), contraction row (k*128 + p).
    zt = nc.dram_tensor("zt", [128, NBLK, 2, KT, 512], bf16,
                        kind="ExternalInput").ap()
    zown = nc.dram_tensor("zown", [128, KT, RPC], bf16,
                          kind="ExternalInput").ap()
    zpr = nc.dram_tensor("zpr", [128, KT, RPC], bf16,
                         kind="ExternalInput").ap()
    out = nc.dram_tensor("out", [1, 1], f32, kind="ExternalOutput").ap()

    with tile.TileContext(nc) as tc:
        with (
            tc.tile_pool(name="sb", bufs=1) as sb,
            tc.tile_pool(name="wrk", bufs=2) as wrk,
            tc.tile_pool(name="wrk1", bufs=1) as wrk1,
            tc.tile_pool(name="psN", bufs=1, space=PSUM) as psN,
            tc.tile_pool(name="psO", bufs=2, space=PSUM) as psO,
            tc.tile_pool(name="psG", bufs=2, space=PSUM) as psG,
        ):
            ones = sb.tile([128, 128], bf16, tag="ones")
            nc.vector.memset(ones[:], 1.0)
            bias_ln16 = sb.tile([128, 1], f32, tag="b16")
            nc.vector.memset(bias_ln16[:], LN_FP8)
            bias_10 = sb.tile([128, 1], f32, tag="b10")
            nc.vector.memset(bias_10[:], SCALE)

            # ---- input DMAs, all on the sync HWDGE queue.  Own slice
            # first (gates the Gram lhs); partner before the last block.
            zok = sb.tile([128, KT, RPC], bf16, tag="zok")
            zpk = sb.tile([128, KT, RPC], bf16, tag="zpk")
            # zb[b] is [128, 2(h), KT, 512]; a half is contiguous 4KB/prt
            zb = [sb.tile([128, 2, KT, 512], bf16, tag=f"zt{b}",
                          name=f"zb{b}") for b in range(NBLK)]
            nc.sync.dma_start(out=zok[:], in_=zown)
            nc.sync.dma_start(out=zb[0][:, 0], in_=zt[:, 0, 0])
            nc.sync.dma_start(out=zb[0][:, 1], in_=zt[:, 0, 1])
            nc.sync.dma_start(out=zb[1][:, 0], in_=zt[:, 1, 0])
            nc.sync.dma_start(out=zb[1][:, 1], in_=zt[:, 1, 1])
            nc.sync.dma_start(out=zb[2][:], in_=zt[:, 2])
            nc.sync.dma_start(out=zpk[:], in_=zpr)
            nc.sync.dma_start(out=zb[3][:], in_=zt[:, 3])

            # ---- PE warmers: keep the clock gate up through the head
            warm = psO.tile([128, 512], f32, tag="pd", name="warmA")
            for _ in range(NWARM_A):
                nc.tensor.matmul(warm[:, 0:128], ones[:], ones[:],
                                 start=True, stop=True)

            # ================= own-slice chain (Gram lhs) =================
            sqo = wrk.tile([128, KT, RPC], bf16, tag="sq_s", name="sqo")
            nc.vector.tensor_tensor(sqo[:], zok[:], zok[:], A.mult)
            pso = psO.tile([128, 512], f32, tag="pd", name="pso")
            for k in range(KT):
                nc.tensor.matmul(pso[:], ones[:], sqo[:, k, :],
                                 start=(k == 0), stop=(k == KT - 1))
            warm2 = psO.tile([128, 512], f32, tag="pd", name="warmB")
            for _ in range(NWARM_B):
                nc.tensor.matmul(warm2[:, 0:128], ones[:], ones[:],
                                 start=True, stop=True)
            lno = wrk.tile([128, RPC], f32, tag="lns_s", name="lno")
            nc.scalar.activation(lno[:], pso[:], F.Ln)
            rino = wrk.tile([128, RPC], bf16, tag="rin_s", name="rino")
            nc.scalar.activation(rino[:], lno[:], F.Exp, scale=-0.5,
                                 bias=bias_ln16[:])
            zno = sb.tile([128, KT, RPC], f8, tag="zno")

            # ================= block normalize conveyor ===================
            zn16 = [sb.tile([128, 2, KT, 512], bf16, tag=f"zn16_{b}",
                            name=f"zn16_{b}") for b in range(NBLK)]
            zn8 = [sb.tile([128, 2, KT, 512], f8, tag=f"zn8_{b}",
                           name=f"zn8_{b}") for b in range(NBLK)]
            rin = [None] * NBLK
            psS = [None] * NBLK
            sq_t = [None] * NBLK

            for b in range(NBLK):
                sq_t[b] = wrk.tile([128, 2, KT, 512], bf16,
                                   tag="sq01" if b < 2 else "sq23",
                                   name=f"sq{b}")
                psS[b] = psN.tile([128, BLK], f32, tag="ssq", name=f"psS{b}")

            def sq_half(b, h):
                nc.vector.tensor_tensor(sq_t[b][:, h], zb[b][:, h],
                                        zb[b][:, h], A.mult)

            def sq_full(b):
                nc.vector.tensor_tensor(sq_t[b][:], zb[b][:], zb[b][:],
                                        A.mult)

            def ssq_half(b, h):
                for k in range(KT):
                    nc.tensor.matmul(psS[b][:, h * 512:(h + 1) * 512],
                                     ones[:], sq_t[b][:, h, k, :],
                                     start=(k == 0), stop=(k == KT - 1))

            def rsqrt_block(b):
                lns = wrk.tile([128, BLK], f32, tag="lns", name=f"lns{b}")
                nc.scalar.activation(lns[:], psS[b][:], F.Ln)
                rin[b] = wrk1.tile([128, 2, 512], bf16, tag=f"rin{b}",
                                   name=f"rin{b}")
                nc.scalar.activation(rin[b][:, 0], lns[:, 0:512],
                                     F.Exp, scale=-0.5, bias=bias_ln16[:])
                nc.scalar.activation(rin[b][:, 1], lns[:, 512:1024],
                                     F.Exp, scale=-0.5, bias=bias_ln16[:])

            def mult_half(b, h):
                nc.vector.tensor_tensor(
                    zn16[b][:, h], zb[b][:, h],
                    rin[b][:, h].unsqueeze(1).broadcast_to([128, KT, 512]),
                    A.mult)
                nc.gpsimd.dma_start(out=zn8[b][:, h], in_=zn16[b][:, h])

            # ---- DVE stream, latency-sorted.  Each instruction appears
            # as soon as its inputs can possibly be ready.
            sq_half(0, 0)
            sq_half(0, 1)
            # zno (Gram lhs) as soon as rino lands
            nc.vector.tensor_tensor(
                zno[:], zok[:],
                rino[:].unsqueeze(1).broadcast_to([128, KT, RPC]), A.mult)
            sq_half(1, 0)
            sq_half(1, 1)

            # PE: ssq accumulations for blocks 0/1
            ssq_half(0, 0)
            ssq_half(0, 1)
            ssq_half(1, 0)
            ssq_half(1, 1)
            rsqrt_block(0)
            rsqrt_block(1)

            mult_half(0, 0)
            mult_half(0, 1)
            sq_full(2)
            mult_half(1, 0)
            mult_half(1, 1)

            # ---------- Gram + fused exp row-sums ----------
            rowp = sb.tile([128, 4, NBLK], f32, tag="rowp")
            scr_n = [0]

            def gram_group(b, m):
                pm = psG.tile([128, BLK], f32, tag="mm", name=f"pm{b}_{m}")
                lhsT0 = zno[:, 0:2, m * 128:(m + 1) * 128]
                lhsT1 = zno[:, 2:4, m * 128:(m + 1) * 128]
                for h in range(2):
                    hs = slice(h * 512, (h + 1) * 512)
                    nc.tensor.matmul(pm[:, hs], lhsT0, zn8[b][:, h, 0:2, :],
                                     start=True, stop=False, perf_mode=DR)
                    nc.tensor.matmul(pm[:, hs], lhsT1, zn8[b][:, h, 2:4, :],
                                     start=False, stop=True, perf_mode=DR)
                scr = wrk.tile([128, BLK], bf16, tag="scr",
                               name=f"scr{scr_n[0]}")
                scr_n[0] += 1
                nc.scalar.activation(
                    scr[:], pm[:], F.Exp,
                    scale=SCALE / (FP8_SCALE ** 2),
                    accum_out=rowp[:, m, b:b + 1])

            for m in range(4):
                gram_group(0, m)

            # block 2/3 + partner conveyor pieces
            ssq_half(2, 0)
            ssq_half(2, 1)
            rsqrt_block(2)
            mult_half(2, 0)
            mult_half(2, 1)

            for m in range(4):
                gram_group(1, m)

            sq_full(3)
            ssq_half(3, 0)
            ssq_half(3, 1)
            rsqrt_block(3)
            mult_half(3, 0)
            mult_half(3, 1)

            # ---- partner norm (bf16 x1): feeds only the finale ----
            sqp = wrk.tile([128, KT, RPC], bf16, tag="sq_s", name="sqp")
            nc.vector.tensor_tensor(sqp[:], zpk[:], zpk[:], A.mult)
            psp = psO.tile([128, 512], f32, tag="pd", name="psp")
            for k in range(KT):
                nc.tensor.matmul(psp[:], ones[:], sqp[:, k, :],
                                 start=(k == 0), stop=(k == KT - 1))
            lnp = wrk.tile([128, RPC], f32, tag="lns_s", name="lnp")
            nc.scalar.activation(lnp[:], psp[:], F.Ln)
            rinp = wrk.tile([128, RPC], bf16, tag="rin_s", name="rinp")
            nc.scalar.activation(rinp[:], lnp[:], F.Exp, scale=-0.5)
            znp = sb.tile([128, KT, RPC], bf16, tag="znp")
            nc.vector.tensor_tensor(
                znp[:], zpk[:],
                rinp[:].unsqueeze(1).broadcast_to([128, KT, RPC]), A.mult)

            for m in range(4):
                gram_group(2, m)
            for m in range(4):
                gram_group(3, m)

            # ---- diagonal recompute (exact fp8-level) + positives ----
            prd = wrk.tile([128, KT, RPC], bf16, tag="prod", name="prd")
            nc.vector.tensor_tensor(prd[:], zno[:], zno[:], A.mult)
            dg = psO.tile([128, 512], f32, tag="pd", name="dg")
            for k in range(KT):
                nc.tensor.matmul(dg[0:1, :], ones[:, 0:1], prd[:, k, :],
                                 start=(k == 0), stop=(k == KT - 1))
            diag_row = sb.tile([1, RPC], bf16, tag="diagrow")
            nc.vector.tensor_scalar_add(diag_row[:], dg[0:1, :],
                                        -FP8_SCALE ** 2)
            dt = psO.tile([128, 512], f32, tag="pd", name="dt")
            for m in range(4):
                nc.tensor.matmul(dt[:, m * 128:(m + 1) * 128],
                                 diag_row[0:1, m * 128:(m + 1) * 128],
                                 ones[0:1, :], start=True, stop=True)
            diag_part = sb.tile([128, 4], f32, tag="diagp")
            for m in range(4):
                nc.vector.tensor_copy(diag_part[:, m:m + 1],
                                      dt[:, m * 128:m * 128 + 1])
            dexp = sb.tile([128, 4], f32, tag="dexp")
            nc.scalar.activation(dexp[:], diag_part[:], F.Exp,
                                 scale=SCALE / (FP8_SCALE ** 2),
                                 bias=bias_10[:])

            prp = wrk.tile([128, KT, RPC], bf16, tag="prod", name="prp")
            nc.vector.tensor_tensor(prp[:], zno[:], znp[:], A.mult)
            pp = psO.tile([128, 512], f32, tag="pd", name="pp")
            for k in range(KT):
                nc.tensor.matmul(pp[:], ones[:], prp[:, k, :],
                                 start=(k == 0), stop=(k == KT - 1))
            pos_red = sb.tile([128, 1], f32, tag="posr")
            nc.vector.tensor_reduce(pos_red[:], pp[:], AX.X, A.add)

            # ---- finale: partial = sum_r ln(Z_r) - 10 * sum_r pos_r ----
            zs = sb.tile([128, 4], f32, tag="zs")
            nc.vector.tensor_reduce(zs[:], rowp[:], AX.X, A.add)
            zarg = sb.tile([128, 4], f32, tag="zarg")
            nc.vector.tensor_tensor(zarg[:], zs[:], dexp[:], A.subtract)
            logz = sb.tile([128, 5], f32, tag="logz")
            nc.scalar.activation(logz[:, 0:4], zarg[:], F.Ln)
            nc.vector.tensor_scalar_mul(
                logz[:, 4:5], pos_red[:], -SCALE / FP8_SCALE / 128.0)
            red1 = sb.tile([128, 1], f32, tag="red1")
            nc.vector.tensor_reduce(red1[:], logz[:], AX.X, A.add)
            fin = sb.tile([1, 1], f32, tag="fin")
            nc.gpsimd.tensor_reduce(fin[:], red1[:], AX.C, A.add)
            nc.sync.dma_start(out=out, in_=fin[:])

    from concourse import bacc as _bacc_mod

    orig_tables = _bacc_mod.get_activation_tables

    def _filtered(arch):
        tables = orig_tables(arch)
        keep = "natural_log_exp_and_others"
        F = mybir.ActivationFunctionType
        if (keep in tables and F.Exp in tables[keep]
                and F.Ln in tables[keep]):
            for name, fns in tables.items():
                if name != keep:
                    fns.discard(F.Exp)
                    fns.discard(F.Ln)
        return tables

    _bacc_mod.get_activation_tables = _filtered
    try:
        nc.compile()
    finally:
        _bacc_mod.get_activation_tables = orig_tables
    return nc


def _get_nc():
    if "nc" not in _CACHE:
        _CACHE["nc"] = _build()
    return _CACHE["nc"]


def _in_maps(z_i, z_j):
    import ml_dtypes

    z = np.concatenate(
        [np.asarray(z_i, np.float32), np.asarray(z_j, np.float32)], axis=0)
    zt = np.ascontiguousarray(z.T).astype(ml_dtypes.bfloat16)
    # [D, N2] -> [128(p), NBLK, 2(h), KT, 512]: per-partition contiguous
    ztH = np.ascontiguousarray(
        zt.reshape(KT, 128, NBLK, 2, 512).transpose(1, 2, 3, 0, 4))

    def slc(off):
        s = zt[:, off:off + RPC]            # [D, RPC]
        return np.ascontiguousarray(s.reshape(KT, 128, RPC).transpose(1, 0, 2))

    maps = []
    for c in range(NCORES):
        o = c * RPC
        po = (o + B) % N2
        maps.append({
            "zt": ztH,
            "zown": slc(o),
            "zpr": slc(po),
        })
    return maps


def _run(z_i, z_j, trace=False):
    from concourse.bass_utils import run_bass_kernel_spmd

    nc = _get_nc()
    return run_bass_kernel_spmd(nc, _in_maps(z_i, z_j), list(range(NCORES)),
                                trace=trace)


def kernel(z_i, z_j):
    res = _run(z_i, z_j, trace=False)
    total = sum(float(r["out"][0, 0]) for r in res.results)
    return np.float32(total / N2)


# revision 7
# speedup vs baseline: 1.2288x; 1.0090x over previous
"""NT-Xent (SimCLR contrastive) loss on Trainium2, sharded across 8 NeuronCores.

Each core computes a [512, 4096] row-slice of the similarity matrix
sim = zn_own^T . zn_all (fp8 DoubleRow matmuls, x16 fp8 scaling), with the
exp row-sums fused into ScalarE's activation accumulator and an exact
fp8-level diagonal recompute. Host sums the 8 scalar partials (the unshard
step). No host arithmetic beyond sharding/layout/dtype-cast of inputs and
summing the per-core partials.

v4 (vs the 62.7us baseline):
  - per-core column permutation: each core's zt is ordered
    [partner 512 | own 512 | rest 3072].  Row-sums are order-invariant, so
    the Gram covers the same set; the own rows' normalize factors are now a
    slice of block 0's rin (bit-identical math, so the diagonal recompute
    still cancels exactly), and the positives read the fp8 partner columns
    straight out of zn8[0].  This deletes the zown/zpr inputs (5.2 -> 4 MiB
    of input DMA) and the entire own/partner normalize chains.
  - half-block normalize conveyor, latency-ordered DVE queue, block-0 first.
  - PE warmers + density keep the clock gate at 2.4 GHz (measured: the Gram
    runs 215ns/matmul warm vs 427ns cold); all bulk elementwise work on DVE
    (GpSimd tensor ops are ~3.5x slower and poison concurrent DVE).
  - one activation-table load: Ln/Exp pinned via the bacc table-map patch.
"""

import numpy as np

B = 2048
D = 512
N2 = 2 * B              # 4096 total rows
NCORES = 8
RPC = N2 // NCORES      # 512 rows per core
KT = D // 128            # 4 contraction tiles
BLK = 1024              # column-block size
NBLK = N2 // BLK        # 4 blocks
TEMP = 0.1
SCALE = 1.0 / TEMP      # 10.0
FP8_SCALE = 16.0        # zn is stored as fp8(zn*16); sim256 = 256*sim
LN_FP8 = float(np.log(FP8_SCALE))
NWARM_A = 28            # PE warmers during the first DMA wait

_CACHE = {}


def _build():
    from concourse import bass, bacc, tile, mybir

    nc = bacc.Bacc("TRN2", target_bir_lowering=False, debug=False,
                   num_devices=NCORES)
    bf16 = mybir.dt.bfloat16
    f32 = mybir.dt.float32
    f8 = mybir.dt.float8e4
    F = mybir.ActivationFunctionType
    A = mybir.AluOpType
    AX = mybir.AxisListType
    DR = mybir.MatmulPerfMode.DoubleRow
    PSUM = bass.MemorySpace.PSUM

    # host-pre-permuted, half-major: zt[p, b, h, k, j] = z^T column
    # perm[b*1024 + h*512 + j], contraction row (k*128 + p), where perm =
    # [partner rows | own rows | rest].
    zt = nc.dram_tensor("zt", [128, NBLK, 2, KT, 512], bf16,
                        kind="ExternalInput").ap()
    out = nc.dram_tensor("out", [1, 1], f32, kind="ExternalOutput").ap()

    with tile.TileContext(nc) as tc:
        with (
            tc.tile_pool(name="sb", bufs=1) as sb,
            tc.tile_pool(name="wrk", bufs=2) as wrk,
            tc.tile_pool(name="wrk1", bufs=1) as wrk1,
            tc.tile_pool(name="psN", bufs=1, space=PSUM) as psN,
            tc.tile_pool(name="psO", bufs=2, space=PSUM) as psO,
            tc.tile_pool(name="psG", bufs=2, space=PSUM) as psG,
        ):
            ones = sb.tile([128, 128], bf16, tag="ones")
            nc.vector.memset(ones[:], 1.0)
            bias_ln16 = sb.tile([128, 1], f32, tag="b16")
            nc.vector.memset(bias_ln16[:], LN_FP8)
            bias_10 = sb.tile([128, 1], f32, tag="b10")
            nc.vector.memset(bias_10[:], SCALE)

            # ---- input DMAs on the sync HWDGE queue; the own half (b0 h1)
            # first since it gates the Gram lhs.
            zb = [sb.tile([128, 2, KT, 512], bf16, tag=f"zt{b}",
                          name=f"zb{b}") for b in range(NBLK)]
            nc.sync.dma_start(out=zb[0][:, 1], in_=zt[:, 0, 1])
            nc.sync.dma_start(out=zb[0][:, 0], in_=zt[:, 0, 0])
            nc.sync.dma_start(out=zb[1][:, 0], in_=zt[:, 1, 0])
            nc.sync.dma_start(out=zb[1][:, 1], in_=zt[:, 1, 1])
            nc.sync.dma_start(out=zb[2][:], in_=zt[:, 2])
            nc.sync.dma_start(out=zb[3][:], in_=zt[:, 3])

            # ---- PE warmers: ramp the clock gate during the DMA head
            warm = psO.tile([128, 512], f32, tag="pd", name="warmA")
            for _ in range(NWARM_A):
                nc.tensor.matmul(warm[:, 0:128], ones[:], ones[:],
                                 start=True, stop=True)

            zn16 = [sb.tile([128, 2, KT, 512], bf16, tag=f"zn16_{b}",
                            name=f"zn16_{b}") for b in range(NBLK)]
            zn8 = [sb.tile([128, 2, KT, 512], f8, tag=f"zn8_{b}",
                           name=f"zn8_{b}") for b in range(NBLK)]
            zno = sb.tile([128, KT, RPC], f8, tag="zno")
            rin = [None] * NBLK
            psS = [None] * NBLK
            sq_t = [None] * NBLK
            for b in range(NBLK):
                sq_t[b] = wrk.tile([128, 2, KT, 512], bf16,
                                   tag="sq01" if b < 2 else "sq23",
                                   name=f"sq{b}")
                psS[b] = psN.tile([128, BLK], f32, tag="ssq", name=f"psS{b}")

            def sq_half(b, h):
                nc.vector.tensor_tensor(sq_t[b][:, h], zb[b][:, h],
                                        zb[b][:, h], A.mult)

            def sq_full(b):
                nc.vector.tensor_tensor(sq_t[b][:], zb[b][:], zb[b][:],
                                        A.mult)

            def ssq_half(b, h):
                for k in range(KT):
                    nc.tensor.matmul(psS[b][:, h * 512:(h + 1) * 512],
                                     ones[:], sq_t[b][:, h, k, :],
                                     start=(k == 0), stop=(k == KT - 1))

            def rsqrt_block(b):
                lns = wrk.tile([128, BLK], f32, tag="lns", name=f"lns{b}")
                nc.scalar.activation(lns[:], psS[b][:], F.Ln)
                rin[b] = wrk1.tile([128, BLK], bf16, tag=f"rin{b}",
                                   name=f"rin{b}")
                nc.scalar.activation(rin[b][:], lns[:], F.Exp, scale=-0.5,
                                     bias=bias_ln16[:])

            def mult_half(b, h):
                nc.vector.tensor_tensor(
                    zn16[b][:, h], zb[b][:, h],
                    rin[b][:, h * 512:(h + 1) * 512]
                    .unsqueeze(1).broadcast_to([128, KT, 512]), A.mult)
                nc.gpsimd.dma_start(out=zn8[b][:, h], in_=zn16[b][:, h])

            # ---- DVE stream, latency-ordered
            sq_half(0, 1)          # own half first
            sq_half(0, 0)
            # PE: block 0 ssq; ScalarE: block 0 rsqrt
            ssq_half(0, 1)
            ssq_half(0, 0)
            rsqrt_block(0)

            # DVE: zno then block 0 normalize multiplies (gate the stream)
            nc.vector.tensor_tensor(
                zno[:], zb[0][:, 1],
                rin[0][:, 512:1024].unsqueeze(1).broadcast_to([128, KT, 512]),
                A.mult)
            mult_half(0, 1)
            mult_half(0, 0)
            sq_half(1, 0)
            sq_half(1, 1)
            ssq_half(1, 0)
            ssq_half(1, 1)
            rsqrt_block(1)

            # ---------- Gram + fused exp row-sums ----------
            rowp = sb.tile([128, 4, NBLK], f32, tag="rowp")
            scr_n = [0]

            def gram_group(b, m):
                pm = psG.tile([128, BLK], f32, tag="mm", name=f"pm{b}_{m}")
                lhsT0 = zno[:, 0:2, m * 128:(m + 1) * 128]
                lhsT1 = zno[:, 2:4, m * 128:(m + 1) * 128]
                for h in range(2):
                    hs = slice(h * 512, (h + 1) * 512)
                    nc.tensor.matmul(pm[:, hs], lhsT0, zn8[b][:, h, 0:2, :],
                                     start=True, stop=False, perf_mode=DR)
                    nc.tensor.matmul(pm[:, hs], lhsT1, zn8[b][:, h, 2:4, :],
                                     start=False, stop=True, perf_mode=DR)
                scr = wrk.tile([128, BLK], bf16, tag="scr",
                               name=f"scr{scr_n[0]}")
                scr_n[0] += 1
                nc.scalar.activation(
                    scr[:], pm[:], F.Exp,
                    scale=SCALE / (FP8_SCALE ** 2),
                    accum_out=rowp[:, m, b:b + 1])

            gram_group(0, 0)
            gram_group(0, 1)

            # block 1 multiplies + block 2 squares while b0 exps run
            mult_half(1, 0)
            mult_half(1, 1)
            sq_full(2)
            ssq_half(2, 0)
            ssq_half(2, 1)
            rsqrt_block(2)

            gram_group(0, 2)
            gram_group(0, 3)
            for m in range(4):
                gram_group(1, m)

            mult_half(2, 0)
            mult_half(2, 1)
            sq_full(3)
            ssq_half(3, 0)
            ssq_half(3, 1)
            rsqrt_block(3)
            mult_half(3, 0)
            mult_half(3, 1)

            for m in range(4):
                gram_group(2, m)
            for m in range(4):
                gram_group(3, m)

            # ---- diagonal recompute (exact fp8-level) + positives ----
            prd = wrk.tile([128, KT, RPC], bf16, tag="prod", name="prd")
            nc.vector.tensor_tensor(prd[:], zno[:], zno[:], A.mult)
            dg = psO.tile([128, 512], f32, tag="pd", name="dg")
            for k in range(KT):
                nc.tensor.matmul(dg[0:1, :], ones[:, 0:1], prd[:, k, :],
                                 start=(k == 0), stop=(k == KT - 1))
            diag_row = sb.tile([1, RPC], bf16, tag="diagrow")
            nc.vector.tensor_scalar_add(diag_row[:], dg[0:1, :],
                                        -FP8_SCALE ** 2)
            dt = psO.tile([128, 512], f32, tag="pd", name="dt")
            for m in range(4):
                nc.tensor.matmul(dt[:, m * 128:(m + 1) * 128],
                                 diag_row[0:1, m * 128:(m + 1) * 128],
                                 ones[0:1, :], start=True, stop=True)
            diag_part = sb.tile([128, 4], f32, tag="diagp")
            for m in range(4):
                nc.vector.tensor_copy(diag_part[:, m:m + 1],
                                      dt[:, m * 128:m * 128 + 1])
            dexp = sb.tile([128, 4], f32, tag="dexp")
            nc.scalar.activation(dexp[:], diag_part[:], F.Exp,
                                 scale=SCALE / (FP8_SCALE ** 2),
                                 bias=bias_10[:])

            # positives: fp8 own x fp8 partner (block 0, half 0)
            prp = wrk.tile([128, KT, RPC], bf16, tag="prod", name="prp")
            nc.vector.tensor_tensor(prp[:], zno[:], zn8[0][:, 0], A.mult)
            pp = psO.tile([128, 512], f32, tag="pd", name="pp")
            for k in range(KT):
                nc.tensor.matmul(pp[:], ones[:], prp[:, k, :],
                                 start=(k == 0), stop=(k == KT - 1))
            pos_red = sb.tile([128, 1], f32, tag="posr")
            nc.vector.tensor_reduce(pos_red[:], pp[:], AX.X, A.add)

            # ---- finale: partial = sum_r ln(Z_r) - 10 * sum_r pos_r ----
            zs = sb.tile([128, 4], f32, tag="zs")
            nc.vector.tensor_reduce(zs[:], rowp[:], AX.X, A.add)
            zarg = sb.tile([128, 4], f32, tag="zarg")
            nc.vector.tensor_tensor(zarg[:], zs[:], dexp[:], A.subtract)
            logz = sb.tile([128, 5], f32, tag="logz")
            nc.scalar.activation(logz[:, 0:4], zarg[:], F.Ln)
            nc.vector.tensor_scalar_mul(
                logz[:, 4:5], pos_red[:], -SCALE / (FP8_SCALE ** 2) / 128.0)
            red1 = sb.tile([128, 1], f32, tag="red1")
            nc.vector.tensor_reduce(red1[:], logz[:], AX.X, A.add)
            fin = sb.tile([1, 1], f32, tag="fin")
            nc.gpsimd.tensor_reduce(fin[:], red1[:], AX.C, A.add)
            nc.sync.dma_start(out=out, in_=fin[:])

    from concourse import bacc as _bacc_mod

    orig_tables = _bacc_mod.get_activation_tables

    def _filtered(arch):
        tables = orig_tables(arch)
        keep = "natural_log_exp_and_others"
        F = mybir.ActivationFunctionType
        if (keep in tables and F.Exp in tables[keep]
                and F.Ln in tables[keep]):
            for name, fns in tables.items():
                if name != keep:
                    fns.discard(F.Exp)
                    fns.discard(F.Ln)
        return tables

    _bacc_mod.get_activation_tables = _filtered
    try:
        nc.compile()
    finally:
        _bacc_mod.get_activation_tables = orig_tables
    return nc


def _get_nc():
    if "nc" not in _CACHE:
        _CACHE["nc"] = _build()
    return _CACHE["nc"]


def _in_maps(z_i, z_j):
    import ml_dtypes

    z = np.concatenate(
        [np.asarray(z_i, np.float32), np.asarray(z_j, np.float32)], axis=0)
    zt = np.ascontiguousarray(z.T).astype(ml_dtypes.bfloat16)  # [D, N2]

    maps = []
    all_idx = np.arange(N2)
    for c in range(NCORES):
        o = c * RPC
        po = (o + B) % N2
        own = all_idx[o:o + RPC]
        par = all_idx[po:po + RPC]
        rest = np.setdiff1d(all_idx, np.concatenate([own, par]))
        perm = np.concatenate([par, own, rest])
        ztp = zt[:, perm]                       # [D, N2], permuted columns
        ztH = np.ascontiguousarray(
            ztp.reshape(KT, 128, NBLK, 2, 512).transpose(1, 2, 3, 0, 4))
        maps.append({"zt": ztH})
    return maps


def _run(z_i, z_j, trace=False):
    from concourse.bass_utils import run_bass_kernel_spmd

    nc = _get_nc()
    return run_bass_kernel_spmd(nc, _in_maps(z_i, z_j), list(range(NCORES)),
                                trace=trace)


def kernel(z_i, z_j):
    res = _run(z_i, z_j, trace=False)
    total = sum(float(r["out"][0, 0]) for r in res.results)
    return np.float32(total / N2)


# revision 8
# speedup vs baseline: 1.2680x; 1.0319x over previous
"""NT-Xent (SimCLR contrastive) loss on Trainium2, sharded across 8 NeuronCores.

Each core computes a [512, 4096] row-slice of the similarity matrix
sim = zn_own^T . zn_all (fp8 DoubleRow matmuls, x16 fp8 scaling), with the
exp row-sums fused into ScalarE's activation accumulator and an exact
fp8-level diagonal recompute. Host sums the 8 scalar partials (the unshard
step). No host arithmetic beyond sharding/layout/dtype-cast of inputs and
summing the per-core partials.

v4 (vs the 62.7us baseline):
  - per-core column permutation: each core's zt is ordered
    [partner 512 | own 512 | rest 3072].  Row-sums are order-invariant, so
    the Gram covers the same set; the own rows' normalize factors are now a
    slice of block 0's rin (bit-identical math, so the diagonal recompute
    still cancels exactly), and the positives read the fp8 partner columns
    straight out of zn8[0].  This deletes the zown/zpr inputs (5.2 -> 4 MiB
    of input DMA) and the entire own/partner normalize chains.
  - half-block normalize conveyor, latency-ordered DVE queue, block-0 first.
  - PE warmers + density keep the clock gate at 2.4 GHz (measured: the Gram
    runs 215ns/matmul warm vs 427ns cold); all bulk elementwise work on DVE
    (GpSimd tensor ops are ~3.5x slower and poison concurrent DVE).
  - one activation-table load: Ln/Exp pinned via the bacc table-map patch.
"""

import numpy as np

B = 2048
D = 512
N2 = 2 * B              # 4096 total rows
NCORES = 8
RPC = N2 // NCORES      # 512 rows per core
KT = D // 128            # 4 contraction tiles
BLK = 1024              # column-block size
NBLK = N2 // BLK        # 4 blocks
TEMP = 0.1
SCALE = 1.0 / TEMP      # 10.0
FP8_SCALE = 16.0        # zn is stored as fp8(zn*16); sim256 = 256*sim
LN_FP8 = float(np.log(FP8_SCALE))
NWARM_A = 28            # PE warmers during the first DMA wait

_CACHE = {}


def _build():
    from concourse import bass, bacc, tile, mybir

    nc = bacc.Bacc("TRN2", target_bir_lowering=False, debug=False,
                   num_devices=NCORES)
    bf16 = mybir.dt.bfloat16
    f32 = mybir.dt.float32
    f8 = mybir.dt.float8e4
    F = mybir.ActivationFunctionType
    A = mybir.AluOpType
    AX = mybir.AxisListType
    DR = mybir.MatmulPerfMode.DoubleRow
    PSUM = bass.MemorySpace.PSUM

    # host-pre-permuted, half-major: zt[p, b, h, k, j] = z^T column
    # perm[b*1024 + h*512 + j], contraction row (k*128 + p), where perm =
    # [partner rows | own rows | rest].
    zt = nc.dram_tensor("zt", [128, NBLK, 2, KT, 512], bf16,
                        kind="ExternalInput").ap()
    out = nc.dram_tensor("out", [1, 1], f32, kind="ExternalOutput").ap()

    with tile.TileContext(nc) as tc:
        with (
            tc.tile_pool(name="sb", bufs=1) as sb,
            tc.tile_pool(name="wrk", bufs=2) as wrk,
            tc.tile_pool(name="wrk1", bufs=1) as wrk1,
            tc.tile_pool(name="psN", bufs=1, space=PSUM) as psN,
            tc.tile_pool(name="psO", bufs=2, space=PSUM) as psO,
            tc.tile_pool(name="psG", bufs=2, space=PSUM) as psG,
        ):
            ones = sb.tile([128, 128], bf16, tag="ones")
            nc.vector.memset(ones[:], 1.0)
            bias_ln16 = sb.tile([128, 1], f32, tag="b16")
            nc.vector.memset(bias_ln16[:], LN_FP8)
            bias_10 = sb.tile([128, 1], f32, tag="b10")
            nc.vector.memset(bias_10[:], SCALE)

            # ---- input DMAs on the sync HWDGE queue; the own half (b0 h1)
            # first since it gates the Gram lhs.
            zb = [sb.tile([128, 2, KT, 512], bf16, tag=f"zt{b}",
                          name=f"zb{b}") for b in range(NBLK)]
            nc.sync.dma_start(out=zb[0][:, 1], in_=zt[:, 0, 1])
            nc.sync.dma_start(out=zb[0][:, 0], in_=zt[:, 0, 0])
            nc.sync.dma_start(out=zb[1][:, 0], in_=zt[:, 1, 0])
            nc.sync.dma_start(out=zb[1][:, 1], in_=zt[:, 1, 1])
            nc.sync.dma_start(out=zb[2][:], in_=zt[:, 2])
            nc.sync.dma_start(out=zb[3][:], in_=zt[:, 3])

            # ---- PE warmers: ramp the clock gate during the DMA head
            warm = psO.tile([128, 512], f32, tag="pd", name="warmA")
            for _ in range(NWARM_A):
                nc.tensor.matmul(warm[:, 0:128], ones[:], ones[:],
                                 start=True, stop=True)

            zn16 = [sb.tile([128, 2, KT, 512], bf16, tag=f"zn16_{b}",
                            name=f"zn16_{b}") for b in range(NBLK)]
            zn8 = [sb.tile([128, 2, KT, 512], f8, tag=f"zn8_{b}",
                           name=f"zn8_{b}") for b in range(NBLK)]
            zno = sb.tile([128, KT, RPC], f8, tag="zno")
            rin = [None] * NBLK
            psS = [None] * NBLK
            sq_t = [None] * NBLK
            for b in range(NBLK):
                sq_t[b] = wrk.tile([128, 2, KT, 512], bf16,
                                   tag="sq01" if b < 2 else "sq23",
                                   name=f"sq{b}")
                psS[b] = psN.tile([128, BLK], f32, tag="ssq", name=f"psS{b}")

            def sq_half(b, h):
                nc.vector.tensor_tensor(sq_t[b][:, h], zb[b][:, h],
                                        zb[b][:, h], A.mult)

            def sq_full(b):
                nc.vector.tensor_tensor(sq_t[b][:], zb[b][:], zb[b][:],
                                        A.mult)

            def ssq_half(b, h):
                for k in range(KT):
                    nc.tensor.matmul(psS[b][:, h * 512:(h + 1) * 512],
                                     ones[:], sq_t[b][:, h, k, :],
                                     start=(k == 0), stop=(k == KT - 1))

            def rsqrt_block(b):
                lns = wrk.tile([128, BLK], f32, tag="lns", name=f"lns{b}")
                nc.scalar.activation(lns[:], psS[b][:], F.Ln)
                rin[b] = wrk1.tile([128, BLK], bf16, tag=f"rin{b}",
                                   name=f"rin{b}")
                nc.scalar.activation(rin[b][:], lns[:], F.Exp, scale=-0.5,
                                     bias=bias_ln16[:])

            def mult_half(b, h):
                nc.vector.tensor_tensor(
                    zn16[b][:, h], zb[b][:, h],
                    rin[b][:, h * 512:(h + 1) * 512]
                    .unsqueeze(1).broadcast_to([128, KT, 512]), A.mult)
                nc.gpsimd.dma_start(out=zn8[b][:, h], in_=zn16[b][:, h])

            # ---- DVE stream, latency-ordered
            sq_half(0, 1)          # own half first
            sq_half(0, 0)
            sq_half(1, 0)
            sq_half(1, 1)
            # PE: ssq; ScalarE: block 0 rsqrt at half granularity so the
            # own-half rin (which gates zno -> the whole Gram) lands first
            ssq_half(0, 1)
            ssq_half(0, 0)
            ssq_half(1, 0)
            ssq_half(1, 1)
            lns0 = wrk.tile([128, BLK], f32, tag="lns", name="lns0")
            rin[0] = wrk1.tile([128, BLK], bf16, tag="rin0", name="rin0")
            nc.scalar.activation(lns0[:, 512:1024], psS[0][:, 512:1024], F.Ln)
            nc.scalar.activation(rin[0][:, 512:1024], lns0[:, 512:1024],
                                 F.Exp, scale=-0.5, bias=bias_ln16[:])
            nc.scalar.activation(lns0[:, 0:512], psS[0][:, 0:512], F.Ln)
            nc.scalar.activation(rin[0][:, 0:512], lns0[:, 0:512],
                                 F.Exp, scale=-0.5, bias=bias_ln16[:])
            rsqrt_block(1)

            # DVE: zno (= the Gram's own-column fp8 too), then block-0
            # partner-half multiply.  zn8[0] h1 IS zno.
            nc.vector.tensor_tensor(
                zno[:], zb[0][:, 1],
                rin[0][:, 512:1024].unsqueeze(1).broadcast_to([128, KT, 512]),
                A.mult)
            mult_half(0, 0)
            # PE clock-keeper fillers: become ready with zno and soak the
            # PE idle window before the Gram so the clock gate stays up.
            fill = psO.tile([128, 512], f32, tag="pd", name="fill")
            for _ in range(8):
                nc.tensor.matmul(fill[:], ones[:], zno[:, 0, :],
                                 start=True, stop=True)

            # ---------- Gram + fused exp row-sums ----------
            rowp = sb.tile([128, 4, NBLK], f32, tag="rowp")
            scr_n = [0]

            def gram_group(b, m):
                pm = psG.tile([128, BLK], f32, tag="mm", name=f"pm{b}_{m}")
                lhsT0 = zno[:, 0:2, m * 128:(m + 1) * 128]
                lhsT1 = zno[:, 2:4, m * 128:(m + 1) * 128]
                for h in range(2):
                    hs = slice(h * 512, (h + 1) * 512)
                    rhs = zno if (b == 0 and h == 1) else zn8[b][:, h]
                    nc.tensor.matmul(pm[:, hs], lhsT0, rhs[:, 0:2, :],
                                     start=True, stop=False, perf_mode=DR)
                    nc.tensor.matmul(pm[:, hs], lhsT1, rhs[:, 2:4, :],
                                     start=False, stop=True, perf_mode=DR)
                scr = wrk.tile([128, BLK], bf16, tag="scr",
                               name=f"scr{scr_n[0]}")
                scr_n[0] += 1
                nc.scalar.activation(
                    scr[:], pm[:], F.Exp,
                    scale=SCALE / (FP8_SCALE ** 2),
                    accum_out=rowp[:, m, b:b + 1])

            # block 1 multiplies can go as soon as rin[1] lands
            mult_half(1, 0)
            mult_half(1, 1)

            gram_group(0, 0)
            gram_group(0, 1)
            gram_group(0, 2)
            gram_group(0, 3)

            sq_full(2)
            ssq_half(2, 0)
            ssq_half(2, 1)
            rsqrt_block(2)
            mult_half(2, 0)
            mult_half(2, 1)

            for m in range(4):
                gram_group(1, m)

            sq_full(3)
            ssq_half(3, 0)
            ssq_half(3, 1)
            rsqrt_block(3)
            mult_half(3, 0)
            mult_half(3, 1)

            for m in range(4):
                gram_group(2, m)
            for m in range(4):
                gram_group(3, m)

            # ---- diagonal recompute (exact fp8-level) + positives ----
            prd = wrk.tile([128, KT, RPC], bf16, tag="prod", name="prd")
            nc.vector.tensor_tensor(prd[:], zno[:], zno[:], A.mult)
            dg = psO.tile([128, 512], f32, tag="pd", name="dg")
            for k in range(KT):
                nc.tensor.matmul(dg[0:1, :], ones[:, 0:1], prd[:, k, :],
                                 start=(k == 0), stop=(k == KT - 1))
            diag_row = sb.tile([1, RPC], bf16, tag="diagrow")
            nc.vector.tensor_scalar_add(diag_row[:], dg[0:1, :],
                                        -FP8_SCALE ** 2)
            dt = psO.tile([128, 512], f32, tag="pd", name="dt")
            for m in range(4):
                nc.tensor.matmul(dt[:, m * 128:(m + 1) * 128],
                                 diag_row[0:1, m * 128:(m + 1) * 128],
                                 ones[0:1, :], start=True, stop=True)
            diag_part = sb.tile([128, 4], f32, tag="diagp")
            for m in range(4):
                nc.vector.tensor_copy(diag_part[:, m:m + 1],
                                      dt[:, m * 128:m * 128 + 1])
            dexp = sb.tile([128, 4], f32, tag="dexp")
            nc.scalar.activation(dexp[:], diag_part[:], F.Exp,
                                 scale=SCALE / (FP8_SCALE ** 2),
                                 bias=bias_10[:])

            # positives: fp8 own x fp8 partner (block 0, half 0)
            prp = wrk.tile([128, KT, RPC], bf16, tag="prod", name="prp")
            nc.vector.tensor_tensor(prp[:], zno[:], zn8[0][:, 0], A.mult)
            pp = psO.tile([128, 512], f32, tag="pd", name="pp")
            for k in range(KT):
                nc.tensor.matmul(pp[:], ones[:], prp[:, k, :],
                                 start=(k == 0), stop=(k == KT - 1))
            pos_red = sb.tile([128, 1], f32, tag="posr")
            nc.vector.tensor_reduce(pos_red[:], pp[:], AX.X, A.add)

            # ---- finale: partial = sum_r ln(Z_r) - 10 * sum_r pos_r ----
            zs = sb.tile([128, 4], f32, tag="zs")
            nc.vector.tensor_reduce(zs[:], rowp[:], AX.X, A.add)
            zarg = sb.tile([128, 4], f32, tag="zarg")
            nc.vector.tensor_tensor(zarg[:], zs[:], dexp[:], A.subtract)
            logz = sb.tile([128, 5], f32, tag="logz")
            nc.scalar.activation(logz[:, 0:4], zarg[:], F.Ln)
            nc.vector.tensor_scalar_mul(
                logz[:, 4:5], pos_red[:], -SCALE / (FP8_SCALE ** 2) / 128.0)
            red1 = sb.tile([128, 1], f32, tag="red1")
            nc.vector.tensor_reduce(red1[:], logz[:], AX.X, A.add)
            fin = sb.tile([1, 1], f32, tag="fin")
            nc.gpsimd.tensor_reduce(fin[:], red1[:], AX.C, A.add)
            nc.sync.dma_start(out=out, in_=fin[:])

    from concourse import bacc as _bacc_mod

    orig_tables = _bacc_mod.get_activation_tables

    def _filtered(arch):
        tables = orig_tables(arch)
        keep = "natural_log_exp_and_others"
        F = mybir.ActivationFunctionType
        if (keep in tables and F.Exp in tables[keep]
                and F.Ln in tables[keep]):
            for name, fns in tables.items():
                if name != keep:
                    fns.discard(F.Exp)
                    fns.discard(F.Ln)
        return tables

    _bacc_mod.get_activation_tables = _filtered
    try:
        nc.compile()
    finally:
        _bacc_mod.get_activation_tables = orig_tables
    return nc


def _get_nc():
    if "nc" not in _CACHE:
        _CACHE["nc"] = _build()
    return _CACHE["nc"]


def _in_maps(z_i, z_j):
    import ml_dtypes

    z = np.concatenate(
        [np.asarray(z_i, np.float32), np.asarray(z_j, np.float32)], axis=0)
    zt = np.ascontiguousarray(z.T).astype(ml_dtypes.bfloat16)  # [D, N2]

    maps = []
    all_idx = np.arange(N2)
    for c in range(NCORES):
        o = c * RPC
        po = (o + B) % N2
        own = all_idx[o:o + RPC]
        par = all_idx[po:po + RPC]
        rest = np.setdiff1d(all_idx, np.concatenate([own, par]))
        perm = np.concatenate([par, own, rest])
        ztp = zt[:, perm]                       # [D, N2], permuted columns
        ztH = np.ascontiguousarray(
            ztp.reshape(KT, 128, NBLK, 2, 512).transpose(1, 2, 3, 0, 4))
        maps.append({"zt": ztH})
    return maps


def _run(z_i, z_j, trace=False):
    from concourse.bass_utils import run_bass_kernel_spmd

    nc = _get_nc()
    return run_bass_kernel_spmd(nc, _in_maps(z_i, z_j), list(range(NCORES)),
                                trace=trace)


def kernel(z_i, z_j):
    res = _run(z_i, z_j, trace=False)
    total = sum(float(r["out"][0, 0]) for r in res.results)
    return np.float32(total / N2)


# revision 9
# speedup vs baseline: 1.3576x; 1.0706x over previous
"""NT-Xent (SimCLR contrastive) loss on Trainium2, sharded across 8 NeuronCores.

Each core computes a [512, 4096] row-slice of the similarity matrix
sim = zn_own^T . zn_all (fp8 DoubleRow matmuls, x16 fp8 scaling), with the
exp row-sums fused into ScalarE's activation accumulator and an exact
fp8-level diagonal recompute. Host sums the 8 scalar partials (the unshard
step). No host arithmetic beyond sharding/layout/dtype-cast of inputs and
summing the per-core partials.

v4 (vs the 62.7us baseline):
  - per-core column permutation: each core's zt is ordered
    [partner 512 | own 512 | rest 3072].  Row-sums are order-invariant, so
    the Gram covers the same set; the own rows' normalize factors are now a
    slice of block 0's rin (bit-identical math, so the diagonal recompute
    still cancels exactly), and the positives read the fp8 partner columns
    straight out of zn8[0].  This deletes the zown/zpr inputs (5.2 -> 4 MiB
    of input DMA) and the entire own/partner normalize chains.
  - half-block normalize conveyor, latency-ordered DVE queue, block-0 first.
  - PE warmers + density keep the clock gate at 2.4 GHz (measured: the Gram
    runs 215ns/matmul warm vs 427ns cold); all bulk elementwise work on DVE
    (GpSimd tensor ops are ~3.5x slower and poison concurrent DVE).
  - one activation-table load: Ln/Exp pinned via the bacc table-map patch.
"""

import numpy as np

B = 2048
D = 512
N2 = 2 * B              # 4096 total rows
NCORES = 8
RPC = N2 // NCORES      # 512 rows per core
KT = D // 128            # 4 contraction tiles
BLK = 1024              # column-block size
NBLK = N2 // BLK        # 4 blocks
TEMP = 0.1
SCALE = 1.0 / TEMP      # 10.0
FP8_SCALE = 16.0        # zn is stored as fp8(zn*16); sim256 = 256*sim
LN_FP8 = float(np.log(FP8_SCALE))
NWARM_A = 28            # PE warmers during the first DMA wait

_CACHE = {}


def _build():
    from concourse import bass, bacc, tile, mybir

    nc = bacc.Bacc("TRN2", target_bir_lowering=False, debug=False,
                   num_devices=NCORES)
    bf16 = mybir.dt.bfloat16
    f32 = mybir.dt.float32
    f8 = mybir.dt.float8e4
    F = mybir.ActivationFunctionType
    A = mybir.AluOpType
    AX = mybir.AxisListType
    DR = mybir.MatmulPerfMode.DoubleRow
    PSUM = bass.MemorySpace.PSUM

    # host-pre-permuted, half-major: zt[p, b, h, k, j] = z^T column
    # perm[b*1024 + h*512 + j], contraction row (k*128 + p), where perm =
    # [partner rows | own rows | rest].
    zt = nc.dram_tensor("zt", [128, NBLK, 2, KT, 512], bf16,
                        kind="ExternalInput").ap()
    out = nc.dram_tensor("out", [1, 1], f32, kind="ExternalOutput").ap()

    with tile.TileContext(nc) as tc:
        with (
            tc.tile_pool(name="sb", bufs=1) as sb,
            tc.tile_pool(name="wrk", bufs=2) as wrk,
            tc.tile_pool(name="wrk1", bufs=1) as wrk1,
            tc.tile_pool(name="psN", bufs=1, space=PSUM) as psN,
            tc.tile_pool(name="psO", bufs=2, space=PSUM) as psO,
            tc.tile_pool(name="psG", bufs=2, space=PSUM) as psG,
        ):
            ones = sb.tile([128, 128], bf16, tag="ones")
            nc.vector.memset(ones[:], 1.0)
            bias_ln16 = sb.tile([128, 1], f32, tag="b16")
            nc.vector.memset(bias_ln16[:], LN_FP8)
            bias_10 = sb.tile([128, 1], f32, tag="b10")
            nc.vector.memset(bias_10[:], SCALE)

            # ---- input DMAs on the sync HWDGE queue; the own half (b0 h1)
            # first since it gates the Gram lhs.
            zb = [sb.tile([128, 2, KT, 512], bf16, tag=f"zt{b}",
                          name=f"zb{b}") for b in range(NBLK)]
            nc.sync.dma_start(out=zb[0][:, 1], in_=zt[:, 0, 1])
            nc.sync.dma_start(out=zb[0][:, 0], in_=zt[:, 0, 0])
            nc.sync.dma_start(out=zb[1][:, 0], in_=zt[:, 1, 0])
            nc.sync.dma_start(out=zb[1][:, 1], in_=zt[:, 1, 1])
            nc.sync.dma_start(out=zb[2][:], in_=zt[:, 2])
            nc.sync.dma_start(out=zb[3][:], in_=zt[:, 3])

            # ---- PE warmers: ramp the clock gate during the DMA head
            warm = psO.tile([128, 512], f32, tag="pd", name="warmA")
            for _ in range(NWARM_A):
                nc.tensor.matmul(warm[:, 0:128], ones[:], ones[:],
                                 start=True, stop=True)

            zn16 = [sb.tile([128, 2, KT, 512], bf16, tag=f"zn16_{b}",
                            name=f"zn16_{b}") for b in range(NBLK)]
            zn8 = [sb.tile([128, 2, KT, 512], f8, tag=f"zn8_{b}",
                           name=f"zn8_{b}") for b in range(NBLK)]
            zno = sb.tile([128, KT, RPC], f8, tag="zno")
            rin = [None] * NBLK
            psS = [None] * NBLK
            sq_t = [None] * NBLK
            for b in range(NBLK):
                sq_t[b] = wrk.tile([128, 2, KT, 512], bf16,
                                   tag="sq01" if b < 2 else "sq23",
                                   name=f"sq{b}")
                psS[b] = psN.tile([128, BLK], f32, tag="ssq", name=f"psS{b}")

            def sq_half(b, h):
                nc.vector.tensor_tensor(sq_t[b][:, h], zb[b][:, h],
                                        zb[b][:, h], A.mult)

            def sq_full(b):
                nc.vector.tensor_tensor(sq_t[b][:], zb[b][:], zb[b][:],
                                        A.mult)

            def ssq_half(b, h):
                for k in range(KT):
                    nc.tensor.matmul(psS[b][:, h * 512:(h + 1) * 512],
                                     ones[:], sq_t[b][:, h, k, :],
                                     start=(k == 0), stop=(k == KT - 1))

            def rsqrt_block(b):
                lns = wrk.tile([128, BLK], f32, tag="lns", name=f"lns{b}")
                nc.scalar.activation(lns[:], psS[b][:], F.Ln)
                rin[b] = wrk1.tile([128, BLK], bf16, tag=f"rin{b}",
                                   name=f"rin{b}")
                nc.scalar.activation(rin[b][:], lns[:], F.Exp, scale=-0.5,
                                     bias=bias_ln16[:])

            def mult_half(b, h):
                nc.vector.tensor_tensor(
                    zn16[b][:, h], zb[b][:, h],
                    rin[b][:, h * 512:(h + 1) * 512]
                    .unsqueeze(1).broadcast_to([128, KT, 512]), A.mult)
                nc.gpsimd.dma_start(out=zn8[b][:, h], in_=zn16[b][:, h])

            # ---- conveyor, latency-ordered.  Block 0's own half gates
            # zno (and with it the whole Gram), so its square/ssq/rsqrt
            # run at half granularity and everything else follows.
            sq_half(0, 1)          # own half first
            sq_half(0, 0)
            ssq_half(0, 1)
            ssq_half(0, 0)
            lns0 = wrk.tile([128, BLK], f32, tag="lns", name="lns0")
            rin[0] = wrk1.tile([128, BLK], bf16, tag="rin0", name="rin0")
            nc.scalar.activation(lns0[:, 512:1024], psS[0][:, 512:1024], F.Ln)
            nc.scalar.activation(rin[0][:, 512:1024], lns0[:, 512:1024],
                                 F.Exp, scale=-0.5, bias=bias_ln16[:])
            nc.scalar.activation(lns0[:, 0:512], psS[0][:, 0:512], F.Ln)
            nc.scalar.activation(rin[0][:, 0:512], lns0[:, 0:512],
                                 F.Exp, scale=-0.5, bias=bias_ln16[:])

            # DVE: zno (= the Gram's own-column fp8 too; zn8[0] h1 IS zno),
            # then the block-0 partner-half multiply.
            nc.vector.tensor_tensor(
                zno[:], zb[0][:, 1],
                rin[0][:, 512:1024].unsqueeze(1).broadcast_to([128, KT, 512]),
                A.mult)
            mult_half(0, 0)
            # PE clock-keeper fillers: become ready with zno and soak the
            # PE idle window before the Gram so the clock gate stays up.
            fill = psO.tile([128, 512], f32, tag="pd", name="fill")
            for _ in range(8):
                nc.tensor.matmul(fill[:], ones[:], zno[:, 0, :],
                                 start=True, stop=True)

            # remaining squares + rsqrts, all ahead of the exp stream in
            # ScalarE priority; sq2 fills DVE's wait for rin1, sq3 runs
            # before block 2's multiplies so no late chain cascades.
            sq_half(1, 0)
            sq_half(1, 1)
            ssq_half(1, 0)
            ssq_half(1, 1)
            rsqrt_block(1)
            sq_full(2)
            ssq_half(2, 0)
            ssq_half(2, 1)
            rsqrt_block(2)
            mult_half(1, 0)
            mult_half(1, 1)
            sq_full(3)
            ssq_half(3, 0)
            ssq_half(3, 1)
            rsqrt_block(3)
            mult_half(2, 0)
            mult_half(2, 1)
            mult_half(3, 0)
            mult_half(3, 1)

            # ---------- Gram + fused exp row-sums ----------
            rowp = sb.tile([128, 4, NBLK], f32, tag="rowp")
            scr_n = [0]

            def gram_group(b, m):
                pm = psG.tile([128, BLK], f32, tag="mm", name=f"pm{b}_{m}")
                lhsT0 = zno[:, 0:2, m * 128:(m + 1) * 128]
                lhsT1 = zno[:, 2:4, m * 128:(m + 1) * 128]
                for h in range(2):
                    hs = slice(h * 512, (h + 1) * 512)
                    rhs = zno if (b == 0 and h == 1) else zn8[b][:, h]
                    nc.tensor.matmul(pm[:, hs], lhsT0, rhs[:, 0:2, :],
                                     start=True, stop=False, perf_mode=DR)
                    nc.tensor.matmul(pm[:, hs], lhsT1, rhs[:, 2:4, :],
                                     start=False, stop=True, perf_mode=DR)
                scr = wrk.tile([128, BLK], bf16, tag="scr",
                               name=f"scr{scr_n[0]}")
                scr_n[0] += 1
                nc.scalar.activation(
                    scr[:], pm[:], F.Exp,
                    scale=SCALE / (FP8_SCALE ** 2),
                    accum_out=rowp[:, m, b:b + 1])

            for b in range(NBLK):
                for m in range(4):
                    gram_group(b, m)

            # ---- diagonal recompute (exact fp8-level) + positives ----
            prd = wrk.tile([128, KT, RPC], bf16, tag="prod", name="prd")
            nc.vector.tensor_tensor(prd[:], zno[:], zno[:], A.mult)
            dg = psO.tile([128, 512], f32, tag="pd", name="dg")
            for k in range(KT):
                nc.tensor.matmul(dg[0:1, :], ones[:, 0:1], prd[:, k, :],
                                 start=(k == 0), stop=(k == KT - 1))
            diag_row = sb.tile([1, RPC], bf16, tag="diagrow")
            nc.vector.tensor_scalar_add(diag_row[:], dg[0:1, :],
                                        -FP8_SCALE ** 2)
            dt = psO.tile([128, 512], f32, tag="pd", name="dt")
            for m in range(4):
                nc.tensor.matmul(dt[:, m * 128:(m + 1) * 128],
                                 diag_row[0:1, m * 128:(m + 1) * 128],
                                 ones[0:1, :], start=True, stop=True)
            diag_part = sb.tile([128, 4], f32, tag="diagp")
            for m in range(4):
                nc.vector.tensor_copy(diag_part[:, m:m + 1],
                                      dt[:, m * 128:m * 128 + 1])
            dexp = sb.tile([128, 4], f32, tag="dexp")
            nc.scalar.activation(dexp[:], diag_part[:], F.Exp,
                                 scale=SCALE / (FP8_SCALE ** 2),
                                 bias=bias_10[:])

            # positives: fp8 own x fp8 partner (block 0, half 0)
            prp = wrk.tile([128, KT, RPC], bf16, tag="prod", name="prp")
            nc.vector.tensor_tensor(prp[:], zno[:], zn8[0][:, 0], A.mult)
            pp = psO.tile([128, 512], f32, tag="pd", name="pp")
            for k in range(KT):
                nc.tensor.matmul(pp[:], ones[:], prp[:, k, :],
                                 start=(k == 0), stop=(k == KT - 1))
            pos_red = sb.tile([128, 1], f32, tag="posr")
            nc.vector.tensor_reduce(pos_red[:], pp[:], AX.X, A.add)

            # ---- finale: partial = sum_r ln(Z_r) - 10 * sum_r pos_r ----
            zs = sb.tile([128, 4], f32, tag="zs")
            nc.vector.tensor_reduce(zs[:], rowp[:], AX.X, A.add)
            zarg = sb.tile([128, 4], f32, tag="zarg")
            nc.vector.tensor_tensor(zarg[:], zs[:], dexp[:], A.subtract)
            logz = sb.tile([128, 5], f32, tag="logz")
            nc.scalar.activation(logz[:, 0:4], zarg[:], F.Ln)
            nc.vector.tensor_scalar_mul(
                logz[:, 4:5], pos_red[:], -SCALE / (FP8_SCALE ** 2) / 128.0)
            red1 = sb.tile([128, 1], f32, tag="red1")
            nc.vector.tensor_reduce(red1[:], logz[:], AX.X, A.add)
            fin = sb.tile([1, 1], f32, tag="fin")
            nc.gpsimd.tensor_reduce(fin[:], red1[:], AX.C, A.add)
            nc.sync.dma_start(out=out, in_=fin[:])

    from concourse import bacc as _bacc_mod

    orig_tables = _bacc_mod.get_activation_tables

    def _filtered(arch):
        tables = orig_tables(arch)
        keep = "natural_log_exp_and_others"
        F = mybir.ActivationFunctionType
        if (keep in tables and F.Exp in tables[keep]
                and F.Ln in tables[keep]):
            for name, fns in tables.items():
                if name != keep:
                    fns.discard(F.Exp)
                    fns.discard(F.Ln)
        return tables

    _bacc_mod.get_activation_tables = _filtered
    try:
        nc.compile()
    finally:
        _bacc_mod.get_activation_tables = orig_tables
    return nc


def _get_nc():
    if "nc" not in _CACHE:
        _CACHE["nc"] = _build()
    return _CACHE["nc"]


def _in_maps(z_i, z_j):
    import ml_dtypes

    z = np.concatenate(
        [np.asarray(z_i, np.float32), np.asarray(z_j, np.float32)], axis=0)
    zt = np.ascontiguousarray(z.T).astype(ml_dtypes.bfloat16)  # [D, N2]

    maps = []
    all_idx = np.arange(N2)
    for c in range(NCORES):
        o = c * RPC
        po = (o + B) % N2
        own = all_idx[o:o + RPC]
        par = all_idx[po:po + RPC]
        rest = np.setdiff1d(all_idx, np.concatenate([own, par]))
        perm = np.concatenate([par, own, rest])
        ztp = zt[:, perm]                       # [D, N2], permuted columns
        ztH = np.ascontiguousarray(
            ztp.reshape(KT, 128, NBLK, 2, 512).transpose(1, 2, 3, 0, 4))
        maps.append({"zt": ztH})
    return maps


def _run(z_i, z_j, trace=False):
    from concourse.bass_utils import run_bass_kernel_spmd

    nc = _get_nc()
    return run_bass_kernel_spmd(nc, _in_maps(z_i, z_j), list(range(NCORES)),
                                trace=trace)


def kernel(z_i, z_j):
    res = _run(z_i, z_j, trace=False)
    total = sum(float(r["out"][0, 0]) for r in res.results)
    return np.float32(total / N2)
